# revision 12
# baseline (speedup 1.0000x reference)
"""Trainium2 Bass kernel for Transformer-XL style relative-position MHSA.

Problem: nn_MultiHeadSelfAttention_14989435863450
  B=2, S=2048, D=512, H=8, dh=64, fp32 I/O.

Sharding (8 cores): core c -> batch b = c//4, head pair h0 = 2*(c%4).
Each core computes its 2 heads' attention and the partial output
projection (out_slice @ Wo[slice]); host sums 4 partials per batch and
adds the constant (bv @ Wo + bo) row vector.

Math folds (exact):
  - bq folds into u,v:  u_eff = (u + bq) / sqrt(D)
  - bk adds a per-query-row constant to scores -> cancels in softmax
  - bv contributes attn-weighted 1 * bv = bv -> host-side constant
  - 1/sqrt(D) folded into q at evacuation time
  - softmax normalization folded into the final per-head output
    projection (per-partition scale on pw), so `at` stays unnormalized.

Relative shift: pos scores stream to a DRAM buffer PB[S, S+1] (fp8)
with rows [0 | posrow_i]; reading PB.flat[S : S + S*S] as [S, S] is
exactly Transformer-XL's pad-reshape-slice shift (incl. the wrap).
The shifted strip is added to the content scores ON THE PE (fp8
identity matmul into the content PSUM); exp reads PSUM directly.

PE scheduling notes (HAM clock-gate):
  - attn-weight transposes go through the DMA xbar (dma_start_transpose,
    SBUF->SBUF), keeping the PE stream pure matmuls so the HAM stays at
    K=8/8 (PE transposes don't count as PE-busy and re-throttle).
  - all dh=64-contraction matmuls are issued in row-group pairs
    (h0 rows 0-63 / h1 rows 64-127) which the PE runs concurrently;
    attn@v h0/h1 are col-group paired (PSUM partitions 0-63/64-127).

All matmul operands bf16/fp8 (fp32 PSUM accumulate); host feeds bf16.
Rel err ~1.5e-3 vs fp32 reference (gate 2e-2).
"""

import math
from contextlib import ExitStack

import numpy as np
import ml_dtypes

import concourse.bass as bass
import concourse.bacc as bacc_mod
import concourse.mybir as mybir
import concourse.tile as tile
from concourse.bass import ts, ds
from concourse.bass_utils import run_bass_kernel_spmd
from concourse.masks import make_identity

FP32 = mybir.dt.float32
BF16 = mybir.dt.bfloat16
FP8 = mybir.dt.float8e4

D_MODEL = 512
NUM_HEADS = 8
D_HEAD = 64
DH2 = 2 * D_HEAD  # head-pair width per core
B_FULL = 2
S_FULL = 2048
P = 128
G = 4  # q-blocks per attn@v group
ISQ = 1.0 / math.sqrt(D_MODEL)

Exp = mybir.ActivationFunctionType.Exp
Copy = mybir.ActivationFunctionType.Copy
ADD = mybir.AluOpType.add
MULT = mybir.AluOpType.mult


def build_nc(S=S_FULL, lookahead=2, at_dma_t=True, pair=True):
    """Build the single-core Bass program (SPMD: same program, 8 cores)."""
    nc = bacc_mod.Bacc()
    NB = S // P          # query blocks
    NK = S // P          # key tiles
    CH = min(512, S)     # score column chunk (PSUM bank)
    NCH = S // CH        # chunks per row
    KD = D_MODEL // P    # contraction tiles over D
    NG = NB // G         # attn@v groups

    xT = nc.declare_dram_parameter("xT", [D_MODEL, S], BF16, isOutput=False)
    posT = nc.declare_dram_parameter("posT", [D_MODEL, S], BF16, isOutput=False)
    Wq = nc.declare_dram_parameter("Wq", [D_MODEL, DH2], BF16, isOutput=False)
    Wk = nc.declare_dram_parameter("Wk", [D_MODEL, DH2], BF16, isOutput=False)
    Wv = nc.declare_dram_parameter("Wv", [D_MODEL, DH2], BF16, isOutput=False)
    Wp = nc.declare_dram_parameter("Wp", [D_MODEL, DH2], BF16, isOutput=False)
    Wo2 = nc.declare_dram_parameter("Wo2", [DH2, D_MODEL], BF16, isOutputFalse=False) if False else nc.declare_dram_parameter("Wo2", [DH2, D_MODEL], BF16, isOutput=False)
    ueff = nc.declare_dram_parameter("ueff", [DH2, 1], FP32, isOutput=False)
    veff = nc.declare_dram_parameter("veff", [DH2, 1], FP32, isOutput=False)
    out_partial = nc.declare_dram_parameter("out_partial", [S, D_MODEL], FP32, isOutput=True)

    with ExitStack() as ctx:
        tc = ctx.enter_context(tile.TileContext(nc))
        consts = ctx.enter_context(tc.tile_pool(name="consts", bufs=1))
        blk = ctx.enter_context(tc.tile_pool(name="blk", bufs=3))
        atp = ctx.enter_context(tc.tile_pool(name="atp", bufs=2))
        dram = ctx.enter_context(tc.tile_pool(name="dram", bufs=1, space="DRAM"))
        psAB = ctx.enter_context(tc.tile_pool(name="psAB", bufs=4 if at_dma_t else 3, space="PSUM"))
        psC = ctx.enter_context(tc.tile_pool(name="psC", bufs=2 if at_dma_t else 1, space="PSUM"))
        psD = ctx.enter_context(tc.tile_pool(name="psD", bufs=2, space="PSUM"))
        psE = ctx.enter_context(tc.tile_pool(name="psE", bufs=2, space="PSUM")) if not at_dma_t else None

        # ---- load constants / inputs ----
        xT_sb = consts.tile([P, KD, S], BF16)
        posT_sb = consts.tile([P, KD, S], BF16)
        for chn in range(NCH):
            nc.sync.dma_start(
                xT_sb[:, :, ts(chn, CH)],
                xT.rearrange("(o p) s -> p o s", p=P)[:, :, ts(chn, CH)],
            )
        for chn in range(NCH):
            nc.sync.dma_start(
                posT_sb[:, :, ts(chn, CH)],
                posT.rearrange("(o p) s -> p o s", p=P)[:, :, ts(chn, CH)],
            )
        w_sbs = {}
        for nm, handle in (("Wq", Wq), ("Wk", Wk), ("Wv", Wv), ("Wp", Wp)):
            w_sb = consts.tile([P, KD, DH2], BF16, name=f"{nm}_sb")
            nc.sync.dma_start(w_sb[:], handle.rearrange("(o p) m -> p o m", p=P))
            w_sbs[nm] = w_sb
        Wo_sb = consts.tile([D_HEAD, 2, D_MODEL], BF16)
        nc.sync.dma_start(Wo_sb[:, 0, :], Wo2[ds(0, D_HEAD), :])
        nc.sync.dma_start(Wo_sb[:, 1, :], Wo2[ds(D_HEAD, D_HEAD), :])
        ident_sb = consts.tile([P, P], BF16)
        make_identity(nc, ident_sb[:])
        ident8_sb = consts.tile([P, P], FP8)
        nc.vector.tensor_copy(ident8_sb[:], ident_sb[:])
        ueff_sb = consts.tile([DH2, 1], FP32)
        nc.sync.dma_start(ueff_sb[:], ueff[:, :])
        veff_sb = consts.tile([DH2, 1], FP32)
        nc.sync.dma_start(veff_sb[:], veff[:, :])
        dveff_sb = consts.tile([DH2, 1], FP32)
        nc.vector.tensor_tensor(
            dveff_sb[:], veff_sb[:], ueff_sb[:], mybir.AluOpType.subtract
        )
        # per-head reciprocal-of-softmax-sums tables [128, NB]
        rec_sb = [consts.tile([P, NB], FP32, name=f"rec{h}") for h in range(2)]

        # ---- projections ----
        # qTu/qTv/kT/pT/vvT: [DH2, S] = W.T @ x  (lhsT = W [D, DH2], rhs = xT)
        qTu = consts.tile([DH2, S], BF16)
        qTv = consts.tile([DH2, S], BF16)
        kT = consts.tile([DH2, S], BF16)
        pT = consts.tile([DH2, S], BF16)
        vvT = consts.tile([DH2, S], BF16)

        def proj_chunks(w_sb, src_sb, emit):
            for chn in range(NCH):
                pq = psAB.tile([P, CH], FP32, tag="psAB", name="pq")
                for kt in range(KD):
                    nc.tensor.matmul(
                        pq[:],
                        lhsT=w_sb[:, kt, :],
                        rhs=src_sb[:, kt, ts(chn, CH)],
                        start=(kt == 0),
                        stop=(kt == KD - 1),
                    )
                emit(chn, pq)

        def emit_q(chn, pq):
            nc.vector.tensor_scalar(qTu[:, ts(chn, CH)], pq[:], ISQ, ueff_sb[:, 0:1], MULT, ADD)
            # qTv from qTu (SBUF) so gpsimd can do it: qTv = qTu + (veff - ueff)
            nc.gpsimd.tensor_scalar(qTv[:, ts(chn, CH)], qTu[:, ts(chn, CH)], 1.0, dveff_sb[:, 0:1], MULT, ADD)

        def mk_emit(dst):
            def emit(chn, pq):
                (nc.scalar.copy if chn % 2 == 0 else nc.vector.tensor_copy)(dst[:, ts(chn, CH)], pq[:])
            return emit

        proj_chunks(w_sbs["Wq"], xT_sb, emit_q)
        proj_chunks(w_sbs["Wk"], xT_sb, mk_emit(kT))
        proj_chunks(w_sbs["Wp"], posT_sb, mk_emit(pT))
        proj_chunks(w_sbs["Wv"], xT_sb, mk_emit(vvT))

        # vv: [S-tiled, DH2] natural layout for attn@v lhsT, via DMA xbar
        vv_sb = consts.tile([P, NK, DH2], BF16)
        if at_dma_t:
            nc.sync.dma_start_transpose(vv_sb[:], vvT[:])
        else:
            for g8 in range(NK // 8):
                pt8 = psE.tile([P, 8, P], BF16, tag="psE", name="vvt8")
                for j in range(8):
                    nc.tensor.transpose(pt8[:, j, :], vvT[:, ts(8 * g8 + j, P)], ident_sb[:])
                eng = nc.vector.tensor_copy if g8 % 2 == 0 else nc.scalar.copy
                eng(vv_sb[:, ds(8 * g8, 8), :], pt8[:])

        # ---- per-head DRAM pos-score buffers (padded for the rel-shift) ----
        PB = [dram.tile([S, S + 1], FP8, name=f"pb{h}") for h in range(2)]

        def pos_block(ib):
            """pos scores for q rows of block ib, both heads (row-paired)."""
            pes = [
                blk.tile([P, S + 1], FP8, tag=f"posext{h}", name="pe")
                for h in range(2)
            ]
            for h in range(2):
                nc.vector.memset(pes[h][:, 0:1], 0.0)
            for chn in range(NCH):
                pps = [psAB.tile([P, CH], FP32, tag="psAB", name="pp") for h in range(2)]
                for h in range(2):
                    nc.tensor.matmul(
                        pps[h][:],
                        lhsT=qTv[ds(h * D_HEAD, D_HEAD), ts(ib, P)],
                        rhs=pT[ds(h * D_HEAD, D_HEAD), ts(chn, CH)],
                        start=True,
                        stop=True,
                    )
                for h in range(2):
                    eng = nc.scalar.copy if (chn + h) % 2 == 0 else nc.vector.tensor_copy
                    eng(pes[h][:, 1 + chn * CH : 1 + (chn + 1) * CH], pps[h][:])
            for h in range(2):
                nc.sync.dma_start(PB[h][ts(ib, P), :], pes[h][:])

        sh_tiles = {}

        def fetch_shift(ib):
            """Prefetch the shifted pos strips for block ib, both heads."""
            for h in range(2):
                sh = blk.tile([P, S], FP8, tag=f"shift{h}", name="sh")
                flat = PB[h].flatten()
                view = flat[ds(S + ib * P * S, P * S)].rearrange("(p s) -> p s", s=S)
                nc.sync.dma_start(sh[:], view)
                sh_tiles[(h, ib)] = sh

        def score_block(ib, atT4s):
            """content + shifted pos (on PE) + exp + xbar transpose, both heads."""
            shs = [sh_tiles.pop((h, ib)) for h in range(2)]
            ats = [blk.tile([P, S], BF16, tag=f"attn{h}", name="at") for h in range(2)]
            sums4s = [blk.tile([P, NCH], FP32, tag=f"sums{h}", name="sums4") for h in range(2)]
            for chn in range(NCH):
                pcs = [psAB.tile([P, CH], FP32, tag="psAB", name="pc") for h in range(2)]
                # per head: content (64-contraction) then the two shifted-pos
                # identity halves (row-group pair); groups close before the
                # next head's group opens.
                for h in range(2):
                    nc.tensor.matmul(
                        pcs[h][:],
                        lhsT=qTu[ds(h * D_HEAD, D_HEAD), ts(ib, P)],
                        rhs=kT[ds(h * D_HEAD, D_HEAD), ts(chn, CH)],
                        start=True,
                        stop=False,
                    )
                    nc.tensor.matmul(
                        pcs[h][:],
                        lhsT=ident8_sb[:],
                        rhs=shs[h][:, ts(chn, CH)],
                        start=False,
                        stop=True,
                        skip_group_check=True,
                    )
                for h in range(2):
                    nc.scalar.activation(
                        ats[h][:, ts(chn, CH)], pcs[h][:], Exp,
                        accum_out=sums4s[h][:, chn : chn + 1],
                    )
            j0 = (ib % G) * P
            for h in range(2):
                # combine partial sums -> reciprocal into rec table
                s2 = blk.tile([P, 2], FP32, tag=f"s2_{h}", name="s2")
                nc.vector.tensor_tensor(s2[:], sums4s[h][:, 0:2], sums4s[h][:, 2:4], ADD)
                s1 = blk.tile([P, 1], FP32, tag=f"s1_{h}", name="s1")
                nc.vector.tensor_tensor(s1[:], s2[:, 0:1], s2[:, 1:2], ADD)
                nc.vector.reciprocal(rec_sb[h][:, ib : ib + 1], s1[:])
                # attn transpose through the DMA xbar (SBUF->SBUF)
                if at_dma_t:
                    nc.sync.dma_start_transpose(atT4s[h][:, :, ds(j0, P)], ats[h][:])
                else:
                    for g8 in range(NK // 8):
                        pt8 = psE.tile([P, 8, P], BF16, tag="psE", name="pt8")
                        for j in range(8):
                            nc.tensor.transpose(pt8[:, j, :], ats[h][:, ts(8 * g8 + j, P)], ident_sb[:])
                        eng = nc.vector.tensor_copy if g8 % 2 == 0 else nc.scalar.copy
                        eng(atT4s[h][:, ds(8 * g8, 8), ds(j0, P)], pt8[:])

        def group_out(g, atT4s):
            """attn@v + output projection for q-blocks [g*G, (g+1)*G)."""
            o2s = []
            for h in range(2):
                po = psC.tile([D_HEAD, G * P], FP32, tag="psC", name="po")
                for kt in range(NK):
                    nc.tensor.matmul(
                        po[:],
                        lhsT=vv_sb[:, kt, ds(h * D_HEAD, D_HEAD)],
                        rhs=atT4s[h][:, kt, :],
                        start=(kt == 0),
                        stop=(kt == NK - 1),
                    )
                o2 = blk.tile([D_HEAD, G * P], BF16, tag=f"o2_{h}", name="o2")
                (nc.scalar.copy if h == 0 else nc.vector.tensor_copy)(o2[:], po[:])
                o2s.append(o2)
            for j in range(G):
                qb = g * G + j
                pws = [psD.tile([P, D_MODEL], FP32, tag="psD", name="pw") for h in range(2)]
                for h in range(2):
                    nc.tensor.matmul(
                        pws[h][:],
                        lhsT=o2s[h][:, ts(j, P)],
                        rhs=Wo_sb[:, h, :],
                        start=True,
                        stop=True,
                    )
                t0 = blk.tile([P, D_MODEL], FP32, tag="t0", name="t0")
                nc.scalar.activation(t0[:], pws[0][:], Copy, scale=rec_sb[0][:, qb : qb + 1])
                fin = blk.tile([P, D_MODEL], FP32, tag="fin", name="fin")
                nc.vector.scalar_tensor_tensor(
                    fin[:], pws[1][:], rec_sb[1][:, qb : qb + 1], t0[:], MULT, ADD
                )
                nc.sync.dma_start(out_partial[ts(qb, P), :], fin[:])

        # ---- main pipeline ----
        # pos runs `lookahead` blocks ahead; the shifted-strip fetch runs 1
        # block ahead; attn@v+projection fire per 4-block group.
        for ib in range(min(lookahead, NB)):
            pos_block(ib)
        fetch_shift(0)
        atT4s_by_g = {}
        for ib in range(NB):
            g = ib // G
            if ib % G == 0:
                atT4s_by_g[g] = [
                    atp.tile([P, NK, G * P], BF16, tag=f"atT{h}", name=f"atT{h}")
                    for h in range(2)
                ]
            if ib + lookahead < NB:
                pos_block(ib + lookahead)
            if ib + 1 < NB:
                fetch_shift(ib + 1)
            score_block(ib, atT4s_by_g[g])
            if ib % G == G - 1:
                group_out(g, atT4s_by_g.pop(g))

    nc.finalize()
    return nc


# ---------------- host side ----------------

_NC_CACHE = {}


import os
def _get_nc(S=S_FULL):
    key = (S, os.environ.get("K_AT_DMA_T", "1"), os.environ.get("K_PAIR", "1"))
    if key not in _NC_CACHE:
        _NC_CACHE[key] = build_nc(S, at_dma_t=key[1] == "1", pair=key[2] == "1")
    return _NC_CACHE[key]


def _bf16(a):
    return np.ascontiguousarray(np.asarray(a, dtype=ml_dtypes.bfloat16))


def make_in_maps(inputs, S=S_FULL, n_cores=8):
    x = np.asarray(inputs["x"], np.float32)
    pos = np.asarray(inputs["pos_embedding"], np.float32)
    Wq = np.asarray(inputs["Wq"], np.float32)
    bq = np.asarray(inputs["bq"], np.float32)
    Wk = np.asarray(inputs["Wk"], np.float32)
    Wv = np.asarray(inputs["Wv"], np.float32)
    Wp = np.asarray(inputs["Wp"], np.float32)
    u = np.asarray(inputs["u"], np.float32)
    v = np.asarray(inputs["v"], np.float32)
    Wo = np.asarray(inputs["Wo"], np.float32)

    xT = [_bf16(x[b, :S].T) for b in range(B_FULL)]
    posT = [_bf16(pos[b, :S].T) for b in range(B_FULL)]
    in_maps = []
    for c in range(n_cores):
        b = c // 4
        h0 = 2 * (c % 4)
        sl = slice(h0 * D_HEAD, (h0 + 2) * D_HEAD)
        u_eff = ((u[h0 : h0 + 2].reshape(-1) + bq[sl]) * ISQ).astype(np.float32)
        v_eff = ((v[h0 : h0 + 2].reshape(-1) + bq[sl]) * ISQ).astype(np.float32)
        in_maps.append(
            {
                "xT": xT[b],
                "posT": posT[b],
                "Wq": _bf16(Wq[:, sl]),
                "Wk": _bf16(Wk[:, sl]),
                "Wv": _bf16(Wv[:, sl]),
                "Wp": _bf16(Wp[:, sl]),
                "Wo2": _bf16(Wo[sl, :]),
                "ueff": u_eff.reshape(DH2, 1),
                "veff": v_eff.reshape(DH2, 1),
            }
        )
    return in_maps


def assemble(inputs, results, S=S_FULL):
    bv = np.asarray(inputs["bv"], np.float64)
    Wo = np.asarray(inputs["Wo"], np.float64)
    bo = np.asarray(inputs["bo"], np.float64)
    const = (bv @ Wo + bo).astype(np.float32)
    out = np.zeros((B_FULL, S, D_MODEL), np.float32)
    for c, res in enumerate(results):
        out[c // 4] += res["out_partial"]
    out += const[None, None, :]
    return out


def _run(inputs, trace=False, **kw):
    nc = _get_nc(S_FULL)
    in_maps = make_in_maps(inputs, S_FULL)
    res = run_bass_kernel_spmd(nc, in_maps, list(range(8)), trace=trace, **kw)
    out = assemble(inputs, res.results, S_FULL)
    return out, res


def kernel(**inputs) -> np.ndarray:
    out, _ = _run(inputs, trace=False)
    return out
